# revision 7
# baseline (speedup 1.0000x reference)
"""DNAEmbedding kernel for 8 Trainium2 NeuronCores (Bass/Tile).

Key observation: with VOCAB=8, every output row
    y[b,s,:] = LN(W @ concat(token_emb[ids[b,s]], dinuc_emb[d_id(ids[b,s], ids[b,s+1])]) + bias)
depends only on the pair (ids[b,s], ids[b,s+1]) -- 64 possible rows -- plus 8
rows for the last position of each sequence (zero dinucleotide part).  So the
whole [32,2048,768] output is a gather from a 72x768 LUT.

Host side: fold the weights into the LUT (f64 -> f32, then split into fp16
hi+lo so the device matmul is fp32-accurate at full PE rate).
Device side (per core, batch-sharded 4 rows/core):
  key[p] = ids[p] + 8*ids[p+1]   (sentinel ids[p+1]:=8 at row boundaries
                                  makes key = 64 + ids[p] exactly the
                                  last-position LUT rows)
  per 128-position tile: one-hot(key) via iota/is_equal -> PE transpose ->
  (onehotT @ lut_hi + onehotT @ lut_lo) accumulated in f32 PSUM -> SBUF ->
  one 6MB DMA per 16 tiles with 48KB contiguous runs per partition.
"""

import os
import numpy as np

import bass_rust
import concourse.bass as bass
import concourse.tile as tile
from concourse import mybir
from concourse.bass_utils import run_bass_kernel_spmd

N_CORES = 8
B, S, H = 32, 2048, 768
DINUC = H // 4                     # 192
ROWS_PER_CORE = B // N_CORES       # 4
POS = ROWS_PER_CORE * S            # 8192 positions per core
P = 128                            # partitions
C = POS // P                       # 64 position-columns  (position = p*C + c)
CHUNK = 16                         # columns per output DMA
N_CHUNKS = C // CHUNK              # 4
LN_EPS = 1e-12

F16 = mybir.dt.float16
F32 = mybir.dt.float32
I32 = mybir.dt.int32

# Results of the last device run (for test harnesses): BassKernelResults.
last_run_results = None


def _build_lut(token_emb, dinuc_emb, proj_W, proj_b, ln_gamma, ln_beta):
    """Fold weights into the 72-row output LUT; return fp16 hi/lo split padded
    to [128, H].  Row k<64: token a=k%8 with next-token b=k//8.  Row 64+v:
    last-position token v (zero dinuc part)."""
    W = proj_W.astype(np.float64)
    A = token_emb.astype(np.float64) @ W[:, :H].T        # [8, H]
    D = dinuc_emb.astype(np.float64) @ W[:, H:].T        # [16, H]
    bias = proj_b.astype(np.float64)

    rows = np.zeros((72, H), dtype=np.float64)
    for k in range(64):
        a, b = k % 8, k // 8
        if a >= 4 and b >= 4:
            d = (a - 4) * 4 + (b - 4)
        else:
            d = 0
        rows[k] = A[a] + D[d] + bias
    for v in range(8):
        rows[64 + v] = A[v] + bias

    mu = rows.mean(axis=-1, keepdims=True)
    var = ((rows - mu) ** 2).mean(axis=-1, keepdims=True)
    lut = (rows - mu) / np.sqrt(var + LN_EPS)
    lut = lut * ln_gamma.astype(np.float64) + ln_beta.astype(np.float64)

    lut32 = np.zeros((P, H), dtype=np.float32)
    lut32[:72] = lut.astype(np.float32)
    lut_hi = lut32.astype(np.float16)
    lut_lo = (lut32 - lut_hi.astype(np.float32)).astype(np.float16)
    return lut_hi, lut_lo


def _split_multiwait(nc):
    """The walrus build in this container rejects >1 sync wait per
    instruction; hoist extra waits onto fresh single-wait EventSemaphore
    instructions inserted just before the original."""
    ctr = 0
    for f in nc.m.functions:
        for blk in f.blocks:
            insts = blk.instructions
            i = 0
            while i < len(insts):
                inst = insts[i]
                si = inst.sync_info
                if si is not None and si.on_wait and len(si.on_wait) > 1:
                    waits = list(si.on_wait)
                    si.on_wait = [waits[-1]]
                    for w in waits[:-1]:
                        ev = mybir.InstEventSemaphore(
                            name=f"I-wsplit-{ctr}", ins=[], outs=[]
                        )
                        ctr += 1
                        ev.engine = inst.engine
                        ev.sync_info = bass_rust.SyncInfo(on_wait=[w], on_update=[])
                        nc.register_instruction(ev)
                        insts.insert(i, ev)
                        i += 1
                i += 1
    return ctr


def build_program(reps: int = 1):
    """Build the per-core Bass program (same program on all 8 cores).

    reps > 1 repeats the whole body (same output region) — benchmarking aid
    to amortize dispatch overhead; the grader path always uses reps=1."""
    nc = bass.Bass("TRN2", target_bir_lowering=False, debug=False,
                   num_devices=N_CORES)

    ids_a = nc.dram_tensor("ids_a", [POS], I32, kind="ExternalInput")
    ids_b = nc.dram_tensor("ids_b", [POS], I32, kind="ExternalInput")
    lut_hi_d = nc.dram_tensor("lut_hi", [P, H], F16, kind="ExternalInput")
    lut_lo_d = nc.dram_tensor("lut_lo", [P, H], F16, kind="ExternalInput")
    ident_d = nc.dram_tensor("ident", [P, P], F16, kind="ExternalInput")
    out = nc.dram_tensor("out", [POS, H], F32, kind="ExternalOutput")

    # out rows viewed as [p, c, h] with row = p*C + c
    out_v = out[:, :].rearrange("(p c) h -> p c h", p=P, c=C)

    with tile.TileContext(nc) as tc:
        with (
            tc.tile_pool(name="const", bufs=1) as cpool,
            tc.tile_pool(name="oh", bufs=4) as ohp,
            tc.tile_pool(name="ohT", bufs=4) as ohtp,
            tc.tile_pool(name="outbuf", bufs=2) as obp,
            tc.tile_pool(name="ps_t", bufs=2, space="PSUM") as pstp,
            tc.tile_pool(name="ps_mm", bufs=3, space="PSUM") as psmp,
        ):
            lut_hi = cpool.tile([P, H], F16)
            lut_lo = cpool.tile([P, H], F16)
            ident = cpool.tile([P, P], F16)
            nc.sync.dma_start(lut_hi[:], lut_hi_d[:, :])
            nc.sync.dma_start(lut_lo[:], lut_lo_d[:, :])
            nc.sync.dma_start(ident[:], ident_d[:, :])

            # iota[p, k] = k  (same on every partition), f32
            iota = cpool.tile([P, P], F32)
            nc.gpsimd.iota(iota[:], pattern=[[1, P]], base=0,
                           channel_multiplier=0,
                           allow_small_or_imprecise_dtypes=True)

            # keys: key(p,c) = ids_a[p*C+c] + 8*ids_b[p*C+c], where ids_b is
            # the next-token stream with sentinel 8 at the last position of
            # each sequence (so key = 64 + ids_a there).
            a_t = cpool.tile([P, C], I32)
            b_t = cpool.tile([P, C], I32)
            nc.sync.dma_start(a_t[:], ids_a[0:POS].rearrange("(p c) -> p c", p=P))
            nc.sync.dma_start(b_t[:], ids_b[0:POS].rearrange("(p c) -> p c", p=P))
            keys_i = cpool.tile([P, C], I32)
            nc.vector.tensor_scalar_mul(keys_i[:], b_t[:], 8)
            nc.vector.tensor_add(keys_i[:], keys_i[:], a_t[:])
            keys_f = cpool.tile([P, C], F32)
            nc.vector.tensor_copy(keys_f[:], keys_i[:])

            for g in range(N_CHUNKS * reps):
                g = g % N_CHUNKS
                out_sb = obp.tile([P, CHUNK * H], F32)
                for cl in range(CHUNK):
                    c = g * CHUNK + cl
                    # one-hot over keys: oh[p, k] = (k == key(p, c))
                    oh = ohp.tile([P, P], F16)
                    nc.vector.tensor_scalar(
                        out=oh[:], in0=iota[:],
                        scalar1=keys_f[:, c:c + 1], scalar2=None,
                        op0=mybir.AluOpType.is_equal,
                    )
                    # transpose -> [k, p] so the matmul contracts over keys
                    ps_t = pstp.tile([P, P], F16)
                    nc.tensor.transpose(ps_t[:], oh[:], ident[:])
                    ohT = ohtp.tile([P, P], F16)
                    if cl % 2 == 0:
                        nc.scalar.copy(ohT[:], ps_t[:])
                    else:
                        nc.vector.tensor_copy(ohT[:], ps_t[:])

                    ps = psmp.tile([P, H], F32)
                    nc.tensor.matmul(ps[:, 0:512], ohT[:], lut_hi[:, 0:512],
                                     start=True, stop=False)
                    nc.tensor.matmul(ps[:, 0:512], ohT[:], lut_lo[:, 0:512],
                                     start=False, stop=True)
                    nc.tensor.matmul(ps[:, 512:H], ohT[:], lut_hi[:, 512:H],
                                     start=True, stop=False)
                    nc.tensor.matmul(ps[:, 512:H], ohT[:], lut_lo[:, 512:H],
                                     start=False, stop=True)

                    dst = out_sb[:, cl * H:(cl + 1) * H]
                    if cl % 2 == 0:
                        nc.vector.tensor_copy(dst, ps[:])
                    else:
                        nc.scalar.copy(dst, ps[:])

                # rows p*C + g*CHUNK + cl: 48KB contiguous per partition
                nc.sync.dma_start(out_v[:, g * CHUNK:(g + 1) * CHUNK, :],
                                  out_sb[:])

    _split_multiwait(nc)
    return nc


_program = None


def kernel(input_ids, token_emb, dinuc_emb, proj_W, proj_b, ln_gamma, ln_beta):
    global _program, last_run_results
    lut_hi, lut_lo = _build_lut(token_emb, dinuc_emb, proj_W, proj_b,
                                ln_gamma, ln_beta)
    ident = np.eye(P, dtype=np.float16)

    in_maps = []
    for i in range(N_CORES):
        ids_rows = np.asarray(input_ids[i * ROWS_PER_CORE:(i + 1) * ROWS_PER_CORE],
                              dtype=np.int32)                    # [4, S]
        ids_next = np.full_like(ids_rows, 8)
        ids_next[:, :-1] = ids_rows[:, 1:]                       # sentinel at S-1
        in_maps.append({
            "ids_a": ids_rows.reshape(-1),
            "ids_b": ids_next.reshape(-1),
            "lut_hi": lut_hi,
            "lut_lo": lut_lo,
            "ident": ident,
        })

    if _program is None:
        _program = build_program()

    trace = os.environ.get("KERNEL_TRACE", "0") == "1"
    res = run_bass_kernel_spmd(_program, in_maps, list(range(N_CORES)),
                               trace=trace)
    last_run_results = res

    out = np.empty((B, S, H), dtype=np.float32)
    for i in range(N_CORES):
        out[i * ROWS_PER_CORE:(i + 1) * ROWS_PER_CORE] = (
            res.results[i]["out"].reshape(ROWS_PER_CORE, S, H))
    return out


# revision 29
# speedup vs baseline: 95.0909x; 95.0909x over previous
"""DNAEmbedding kernel for 8 Trainium2 NeuronCores (Bass/Tile).

Key observation: with VOCAB=8, every output row
    y[b,s,:] = LN(W @ concat(token_emb[ids[b,s]], dinuc_emb[d_id(ids[b,s], ids[b,s+1])]) + bias)
depends only on the pair (ids[b,s], ids[b,s+1]) -- 64 possible rows -- plus 8
rows for the last position of each sequence (zero dinucleotide part).  So the
whole [32,2048,768] output is a gather from a 72x768 LUT.

Host side: fold the weights into the LUT (f64 -> f32, then split into fp16
hi+lo so the device matmul is fp32-accurate at full PE rate).
Device side (per core, batch-sharded 4 rows/core):
  key[p] = ids[p] + 8*ids[p+1]   (sentinel ids[p+1]:=8 at row boundaries
                                  makes key = 64 + ids[p] exactly the
                                  last-position LUT rows)
  per 128-position tile: one-hot(key) via iota/is_equal -> PE transpose ->
  (onehotT @ lut_hi + onehotT @ lut_lo) accumulated in f32 PSUM -> SBUF ->
  one 6MB DMA per 16 tiles with 48KB contiguous runs per partition.
"""

import os
import numpy as np

import bass_rust
import concourse.bass as bass
import concourse.tile as tile
from concourse import mybir
from concourse.bass_utils import run_bass_kernel_spmd

N_CORES = 8
B, S, H = 32, 2048, 768
DINUC = H // 4                     # 192
ROWS_PER_CORE = B // N_CORES       # 4
POS = ROWS_PER_CORE * S            # 8192 positions per core
P = 128                            # partitions
C = POS // P                       # 64 position-columns  (position = p*C + c)
CHUNK = int(os.environ.get("KERNEL_CHUNK", "8"))   # columns per output DMA
N_CHUNKS = C // CHUNK
OUT_BUFS = int(os.environ.get("KERNEL_OUT_BUFS", "3"))
# widths of the leading ramp-up chunks (may be trimmed to fit C)
RAMP_CHUNKS = tuple(
    int(x) for x in os.environ.get("KERNEL_RAMP", "").split(",") if x)
WARMUP_MM = int(os.environ.get("KERNEL_WARMUP_MM", "6"))
ALT_RINGS = os.environ.get("KERNEL_ALT_RINGS", "1") == "1"
LN_EPS = 1e-12

F16 = mybir.dt.float16
F32 = mybir.dt.float32
I32 = mybir.dt.int32

# Results of the last device run (for test harnesses): BassKernelResults.
last_run_results = None


def _build_lut(token_emb, dinuc_emb, proj_W, proj_b, ln_gamma, ln_beta):
    """Fold weights into the 72-row output LUT; return fp16 hi/lo split padded
    to [128, H].  Row k<64: token a=k%8 with next-token b=k//8.  Row 64+v:
    last-position token v (zero dinuc part)."""
    W = proj_W.astype(np.float64)
    A = token_emb.astype(np.float64) @ W[:, :H].T        # [8, H]
    D = dinuc_emb.astype(np.float64) @ W[:, H:].T        # [16, H]
    bias = proj_b.astype(np.float64)

    rows = np.zeros((72, H), dtype=np.float64)
    for k in range(64):
        a, b = k % 8, k // 8
        if a >= 4 and b >= 4:
            d = (a - 4) * 4 + (b - 4)
        else:
            d = 0
        rows[k] = A[a] + D[d] + bias
    for v in range(8):
        rows[64 + v] = A[v] + bias

    mu = rows.mean(axis=-1, keepdims=True)
    var = ((rows - mu) ** 2).mean(axis=-1, keepdims=True)
    lut = (rows - mu) / np.sqrt(var + LN_EPS)
    lut = lut * ln_gamma.astype(np.float64) + ln_beta.astype(np.float64)

    lut32 = np.zeros((P, H), dtype=np.float32)
    lut32[:72] = lut.astype(np.float32)
    lut_hi = lut32.astype(np.float16)
    lut_lo = (lut32 - lut_hi.astype(np.float32)).astype(np.float16)
    return lut_hi, lut_lo


def _split_multiwait(nc):
    """The walrus build in this container rejects >1 sync wait per
    instruction; hoist extra waits onto fresh single-wait EventSemaphore
    instructions inserted just before the original."""
    ctr = 0
    for f in nc.m.functions:
        for blk in f.blocks:
            insts = blk.instructions
            i = 0
            while i < len(insts):
                inst = insts[i]
                si = inst.sync_info
                if si is not None and si.on_wait and len(si.on_wait) > 1:
                    waits = list(si.on_wait)
                    si.on_wait = [waits[-1]]
                    for w in waits[:-1]:
                        ev = mybir.InstEventSemaphore(
                            name=f"I-wsplit-{ctr}", ins=[], outs=[]
                        )
                        ctr += 1
                        ev.engine = inst.engine
                        ev.sync_info = bass_rust.SyncInfo(on_wait=[w], on_update=[])
                        nc.register_instruction(ev)
                        insts.insert(i, ev)
                        i += 1
                i += 1
    return ctr


def build_program(reps: int = 1):
    """Build the per-core Bass program (same program on all 8 cores).

    reps > 1 repeats the whole body (same output region) — benchmarking aid
    to amortize dispatch overhead; the grader path always uses reps=1."""
    nc = bass.Bass("TRN2", target_bir_lowering=False, debug=False,
                   num_devices=N_CORES)

    # ids arrive c-major ([C, P]: element (c, j) = position j*C + c) so keys
    # land with c on partitions, ready for the partition-0 flatten
    ids_a = nc.dram_tensor("ids_a", [C, P], I32, kind="ExternalInput")
    ids_b = nc.dram_tensor("ids_b", [C, P], I32, kind="ExternalInput")
    lut_hi_d = nc.dram_tensor("lut_hi", [P, H], F16, kind="ExternalInput")
    lut_lo_d = nc.dram_tensor("lut_lo", [P, H], F16, kind="ExternalInput")
    out = nc.dram_tensor("out", [POS, H], F32, kind="ExternalOutput")

    # out rows viewed as [p, c, h] with row = p*C + c
    out_v = out[:, :].rearrange("(p c) h -> p c h", p=P, c=C)

    with tile.TileContext(nc) as tc:
        with (
            tc.tile_pool(name="const", bufs=1) as cpool,
            tc.tile_pool(name="ohT", bufs=6) as ohtp,
            tc.tile_pool(name="outbuf", bufs=OUT_BUFS) as obp,
            tc.tile_pool(name="ps_kb", bufs=2, space="PSUM") as pskb,
            tc.tile_pool(name="ps_mm", bufs=3, space="PSUM") as psmp,
        ):
            # ids first on the SP ring (keys are the critical path); LUTs on
            # the ACT ring so the two input streams don't serialize
            a_t = cpool.tile([C, P], I32)
            b_t = cpool.tile([C, P], I32)
            nc.sync.dma_start(a_t[:], ids_a[:, :])
            nc.sync.dma_start(b_t[:], ids_b[:, :])
            lut_hi = cpool.tile([P, H], F16)
            lut_lo = cpool.tile([P, H], F16)
            nc.scalar.dma_start(lut_hi[:], lut_hi_d[:, :])
            nc.scalar.dma_start(lut_lo[:], lut_lo_d[:, :])

            # iota[k, j] = k  (constant along free dim), f32
            iota = cpool.tile([P, P], F32)
            nc.gpsimd.iota(iota[:], pattern=[[0, P]], base=0,
                           channel_multiplier=1,
                           allow_small_or_imprecise_dtypes=True)
            ones = cpool.tile([1, P], F16)
            nc.vector.memset(ones[:], 1.0)

            # warm the PE clock gate while the keys chain is in flight
            for w in range(WARMUP_MM):
                wp = psmp.tile([P, H], F32, tag="ps")
                nc.tensor.matmul(wp[:, 0:P], iota[:], iota[:],
                                 start=True, stop=True)

            # keys: key = ids_a + 8*ids_b, where ids_b is the next-token
            # stream with sentinel 8 at the last position of each sequence
            # (so key = 64 + ids_a there).
            keys_i = cpool.tile([C, P], I32)
            nc.vector.tensor_scalar_mul(keys_i[:], b_t[:], 8)
            nc.vector.tensor_add(keys_i[:], keys_i[:], a_t[:])
            keys_t = cpool.tile([C, P], F16)
            nc.vector.tensor_copy(keys_t[:], keys_i[:])
            # flatten to partition 0 (c-major) so every tile's key row is a
            # [1, 128] slice with base partition 0 (matmul alignment rule)
            keys_row = cpool.tile([1, C * P], F16)
            nc.sync.dma_start(keys_row[:], keys_t[:, :])

            # small chunks first so the output-DMA pipeline starts early
            chunks = []
            rem = C
            for w in RAMP_CHUNKS:
                if rem - w < CHUNK:
                    break
                chunks.append(w)
                rem -= w
            while rem > 0:
                chunks.append(min(CHUNK, rem))
                rem -= min(CHUNK, rem)
            starts = [sum(chunks[:i]) for i in range(len(chunks))]

            for gi in range(len(chunks) * reps):
                gi = gi % len(chunks)
                width, c0 = chunks[gi], starts[gi]
                out_sb = obp.tile([P, width * H], F32, tag="out_sb")
                for cl in range(width):
                    c = c0 + cl
                    # replicate tile-c keys across partitions via K=1 matmul:
                    # kb[m, j] = key(j*C + c) for every partition m
                    kb = pskb.tile([P, P], F32, tag="kb")
                    nc.tensor.matmul(kb[:], ones[:],
                                     keys_row[0:1, c * P:(c + 1) * P],
                                     start=True, stop=True)
                    # one-hot already in [key, pos] orientation
                    ohT = ohtp.tile([P, P], F16)
                    nc.vector.tensor_tensor(out=ohT[:], in0=iota[:], in1=kb[:],
                                            op=mybir.AluOpType.is_equal)

                    ps = psmp.tile([P, H], F32, tag="ps")
                    nc.tensor.matmul(ps[:, 0:512], ohT[:], lut_hi[:, 0:512],
                                     start=True, stop=False)
                    nc.tensor.matmul(ps[:, 0:512], ohT[:], lut_lo[:, 0:512],
                                     start=False, stop=True)
                    nc.tensor.matmul(ps[:, 512:H], ohT[:], lut_hi[:, 512:H],
                                     start=True, stop=False)
                    nc.tensor.matmul(ps[:, 512:H], ohT[:], lut_lo[:, 512:H],
                                     start=False, stop=True)

                    dst = out_sb[:, cl * H:(cl + 1) * H]
                    # ACT copies are ~12% slower: give ACT 5 of 8, DVE 3 of 8
                    if cl % 8 in (0, 3, 6):
                        nc.vector.tensor_copy(dst, ps[:])
                    else:
                        nc.scalar.copy(dst, ps[:])

                # rows p*C + c0 + cl: contiguous run per partition
                eng = nc.scalar if (ALT_RINGS and gi % 2) else nc.sync
                eng.dma_start(out_v[:, c0:c0 + width, :], out_sb[:])

    _split_multiwait(nc)
    return nc


_program = None


def kernel(input_ids, token_emb, dinuc_emb, proj_W, proj_b, ln_gamma, ln_beta):
    global _program, last_run_results
    lut_hi, lut_lo = _build_lut(token_emb, dinuc_emb, proj_W, proj_b,
                                ln_gamma, ln_beta)

    in_maps = []
    for i in range(N_CORES):
        ids_rows = np.asarray(input_ids[i * ROWS_PER_CORE:(i + 1) * ROWS_PER_CORE],
                              dtype=np.int32)                    # [4, S]
        ids_next = np.full_like(ids_rows, 8)
        ids_next[:, :-1] = ids_rows[:, 1:]                       # sentinel at S-1
        # c-major layout: element (c, j) = flat position j*C + c
        to_cm = lambda a: np.ascontiguousarray(a.reshape(P, C).T)
        in_maps.append({
            "ids_a": to_cm(ids_rows.reshape(-1)),
            "ids_b": to_cm(ids_next.reshape(-1)),
            "lut_hi": lut_hi,
            "lut_lo": lut_lo,
        })

    if _program is None:
        _program = build_program()

    trace = os.environ.get("KERNEL_TRACE", "0") == "1"
    res = run_bass_kernel_spmd(_program, in_maps, list(range(N_CORES)),
                               trace=trace)
    last_run_results = res

    out = np.empty((B, S, H), dtype=np.float32)
    for i in range(N_CORES):
        out[i * ROWS_PER_CORE:(i + 1) * ROWS_PER_CORE] = (
            res.results[i]["out"].reshape(ROWS_PER_CORE, S, H))
    return out


# revision 33
# speedup vs baseline: 103.2234x; 1.0855x over previous
"""DNAEmbedding kernel for 8 Trainium2 NeuronCores (Bass/Tile).

Key observation: with VOCAB=8, every output row
    y[b,s,:] = LN(W @ concat(token_emb[ids[b,s]], dinuc_emb[d_id(ids[b,s], ids[b,s+1])]) + bias)
depends only on the pair (ids[b,s], ids[b,s+1]) -- 64 possible rows -- plus 8
rows for the last position of each sequence (zero dinucleotide part).  So the
whole [32,2048,768] output is a gather from a 72x768 LUT.

Host side: fold the weights into the LUT (f64 -> f32, then split into fp16
hi+lo so the device matmul is fp32-accurate at full fp16 PE rate).
Device side (per core, batch-sharded 4 rows/core, position = p*64 + c):
  key[s] = ids[s] + 8*ids[s+1]   (sentinel next-token := 8 at sequence ends
                                  makes key = 64 + ids[s], exactly the
                                  last-position LUT rows; no collisions)
  per 128-position tile c: a K=1 matmul (ones[1,128] x key row) replicates
  the tile's keys across partitions in PSUM; is_equal against a channel-iota
  yields the one-hot directly in [key, pos] orientation; 4 fp16 matmuls
  (hi/lo x N=512/256) accumulate the fp32 rows in PSUM; DVE/ACT alternate
  the PSUM->SBUF copies; one 3MB DMA per 8 tiles writes the output with
  24KB-contiguous runs per partition.  Cost-model timeline: ~92.5us/core
  (output-DMA floor ~67us at the 368GB/s HBM derate).
"""

import os
import numpy as np

import bass_rust
import concourse.bass as bass
import concourse.tile as tile
from concourse import mybir
from concourse.bass_utils import run_bass_kernel_spmd

N_CORES = 8
B, S, H = 32, 2048, 768
DINUC = H // 4                     # 192
ROWS_PER_CORE = B // N_CORES       # 4
POS = ROWS_PER_CORE * S            # 8192 positions per core
P = 128                            # partitions
C = POS // P                       # 64 position-columns  (position = p*C + c)
CHUNK = int(os.environ.get("KERNEL_CHUNK", "4"))   # columns per output DMA
N_CHUNKS = C // CHUNK
OUT_BUFS = int(os.environ.get("KERNEL_OUT_BUFS", "6"))
# widths of the leading ramp-up chunks (may be trimmed to fit C)
RAMP_CHUNKS = tuple(
    int(x) for x in os.environ.get("KERNEL_RAMP", "1,2,3").split(",") if x)
WARMUP_MM = int(os.environ.get("KERNEL_WARMUP_MM", "6"))
ALT_RINGS = os.environ.get("KERNEL_ALT_RINGS", "1") == "1"
LN_EPS = 1e-12

F16 = mybir.dt.float16
F32 = mybir.dt.float32
I32 = mybir.dt.int32

# Results of the last device run (for test harnesses): BassKernelResults.
last_run_results = None


def _build_lut(token_emb, dinuc_emb, proj_W, proj_b, ln_gamma, ln_beta):
    """Fold weights into the 72-row output LUT; return fp16 hi/lo split padded
    to [128, H].  Row k<64: token a=k%8 with next-token b=k//8.  Row 64+v:
    last-position token v (zero dinuc part)."""
    W = proj_W.astype(np.float64)
    A = token_emb.astype(np.float64) @ W[:, :H].T        # [8, H]
    D = dinuc_emb.astype(np.float64) @ W[:, H:].T        # [16, H]
    bias = proj_b.astype(np.float64)

    rows = np.zeros((72, H), dtype=np.float64)
    for k in range(64):
        a, b = k % 8, k // 8
        if a >= 4 and b >= 4:
            d = (a - 4) * 4 + (b - 4)
        else:
            d = 0
        rows[k] = A[a] + D[d] + bias
    for v in range(8):
        rows[64 + v] = A[v] + bias

    mu = rows.mean(axis=-1, keepdims=True)
    var = ((rows - mu) ** 2).mean(axis=-1, keepdims=True)
    lut = (rows - mu) / np.sqrt(var + LN_EPS)
    lut = lut * ln_gamma.astype(np.float64) + ln_beta.astype(np.float64)

    lut32 = np.zeros((P, H), dtype=np.float32)
    lut32[:72] = lut.astype(np.float32)
    lut_hi = lut32.astype(np.float16)
    lut_lo = (lut32 - lut_hi.astype(np.float32)).astype(np.float16)
    return lut_hi, lut_lo


def _split_multiwait(nc):
    """The walrus build in this container rejects >1 sync wait per
    instruction; hoist extra waits onto fresh single-wait EventSemaphore
    instructions inserted just before the original."""
    ctr = 0
    for f in nc.m.functions:
        for blk in f.blocks:
            insts = blk.instructions
            i = 0
            while i < len(insts):
                inst = insts[i]
                si = inst.sync_info
                if si is not None and si.on_wait and len(si.on_wait) > 1:
                    waits = list(si.on_wait)
                    si.on_wait = [waits[-1]]
                    for w in waits[:-1]:
                        ev = mybir.InstEventSemaphore(
                            name=f"I-wsplit-{ctr}", ins=[], outs=[]
                        )
                        ctr += 1
                        ev.engine = inst.engine
                        ev.sync_info = bass_rust.SyncInfo(on_wait=[w], on_update=[])
                        nc.register_instruction(ev)
                        insts.insert(i, ev)
                        i += 1
                i += 1
    return ctr


def build_program(reps: int = 1):
    """Build the per-core Bass program (same program on all 8 cores).

    reps > 1 repeats the whole body (same output region) — benchmarking aid
    to amortize dispatch overhead; the grader path always uses reps=1."""
    nc = bass.Bass("TRN2", target_bir_lowering=False, debug=False,
                   num_devices=N_CORES)

    # ids arrive c-major ([C, P]: element (c, j) = position j*C + c) so keys
    # land with c on partitions, ready for the partition-0 flatten
    ids_a = nc.dram_tensor("ids_a", [C, P], I32, kind="ExternalInput")
    ids_b = nc.dram_tensor("ids_b", [C, P], I32, kind="ExternalInput")
    lut_hi_d = nc.dram_tensor("lut_hi", [P, H], F16, kind="ExternalInput")
    lut_lo_d = nc.dram_tensor("lut_lo", [P, H], F16, kind="ExternalInput")
    out = nc.dram_tensor("out", [POS, H], F32, kind="ExternalOutput")

    # out rows viewed as [p, c, h] with row = p*C + c
    out_v = out[:, :].rearrange("(p c) h -> p c h", p=P, c=C)

    with tile.TileContext(nc) as tc:
        with (
            tc.tile_pool(name="const", bufs=1) as cpool,
            tc.tile_pool(name="ohT", bufs=6) as ohtp,
            tc.tile_pool(name="outbuf", bufs=OUT_BUFS) as obp,
            tc.tile_pool(name="ps_kb", bufs=3, space="PSUM") as pskb,
            tc.tile_pool(name="ps_mm", bufs=2, space="PSUM") as psmp,
        ):
            # ids first on the SP ring (keys are the critical path); LUTs on
            # the ACT ring so the two input streams don't serialize
            a_t = cpool.tile([C, P], I32)
            b_t = cpool.tile([C, P], I32)
            nc.sync.dma_start(a_t[:], ids_a[:, :])
            nc.sync.dma_start(b_t[:], ids_b[:, :])
            lut_hi = cpool.tile([P, H], F16)
            lut_lo = cpool.tile([P, H], F16)
            nc.scalar.dma_start(lut_hi[:], lut_hi_d[:, :])
            nc.scalar.dma_start(lut_lo[:], lut_lo_d[:, :])

            # iota[k, j] = k  (constant along free dim), f32
            iota = cpool.tile([P, P], F32)
            nc.gpsimd.iota(iota[:], pattern=[[0, P]], base=0,
                           channel_multiplier=1,
                           allow_small_or_imprecise_dtypes=True)
            ones = cpool.tile([1, P], F16)
            nc.vector.memset(ones[:], 1.0)

            # warm the PE clock gate while the keys chain is in flight
            for _ in range(WARMUP_MM):
                wp = psmp.tile([P, H], F32, tag="ps")
                nc.tensor.matmul(wp[:, 0:P], iota[:], iota[:],
                                 start=True, stop=True)

            # keys: key = ids_a + 8*ids_b, where ids_b is the next-token
            # stream with sentinel 8 at the last position of each sequence
            # (so key = 64 + ids_a there).
            keys_i = cpool.tile([C, P], I32)
            nc.vector.tensor_scalar_mul(keys_i[:], b_t[:], 8)
            nc.vector.tensor_add(keys_i[:], keys_i[:], a_t[:])
            keys_t = cpool.tile([C, P], F16)
            nc.vector.tensor_copy(keys_t[:], keys_i[:])
            # flatten to partition 0 (c-major) so every tile's key row is a
            # [1, 128] slice with base partition 0 (matmul alignment rule)
            keys_row = cpool.tile([1, C * P], F16)
            nc.sync.dma_start(keys_row[:], keys_t[:, :])

            # small chunks first so the output-DMA pipeline starts early
            chunks = []
            rem = C
            for w in RAMP_CHUNKS:
                if rem - w < CHUNK:
                    break
                chunks.append(w)
                rem -= w
            while rem > 0:
                chunks.append(min(CHUNK, rem))
                rem -= min(CHUNK, rem)
            starts = [sum(chunks[:i]) for i in range(len(chunks))]

            for gi in range(len(chunks) * reps):
                gi = gi % len(chunks)
                width, c0 = chunks[gi], starts[gi]
                out_sb = obp.tile([P, width * H], F32, tag="out_sb")
                for cl in range(width):
                    c = c0 + cl
                    # replicate tile-c keys across partitions via K=1 matmul:
                    # kb[m, j] = key(j*C + c) for every partition m
                    kb = pskb.tile([P, P], F32, tag="kb")
                    nc.tensor.matmul(kb[:], ones[:],
                                     keys_row[0:1, c * P:(c + 1) * P],
                                     start=True, stop=True)
                    # one-hot already in [key, pos] orientation
                    ohT = ohtp.tile([P, P], F16)
                    nc.vector.tensor_tensor(out=ohT[:], in0=iota[:], in1=kb[:],
                                            op=mybir.AluOpType.is_equal)

                    ps = psmp.tile([P, H], F32, tag="ps")
                    nc.tensor.matmul(ps[:, 0:512], ohT[:], lut_hi[:, 0:512],
                                     start=True, stop=False)
                    nc.tensor.matmul(ps[:, 0:512], ohT[:], lut_lo[:, 0:512],
                                     start=False, stop=True)
                    nc.tensor.matmul(ps[:, 512:H], ohT[:], lut_hi[:, 512:H],
                                     start=True, stop=False)
                    nc.tensor.matmul(ps[:, 512:H], ohT[:], lut_lo[:, 512:H],
                                     start=False, stop=True)

                    dst = out_sb[:, cl * H:(cl + 1) * H]
                    # ACT copies are ~12% slower: give ACT 5 of 8, DVE 3 of 8
                    if cl % 8 in (0, 3, 6):
                        nc.vector.tensor_copy(dst, ps[:])
                    else:
                        nc.scalar.copy(dst, ps[:])

                # rows p*C + c0 + cl: contiguous run per partition
                eng = nc.scalar if (ALT_RINGS and gi % 2) else nc.sync
                eng.dma_start(out_v[:, c0:c0 + width, :], out_sb[:])

    _split_multiwait(nc)
    return nc


_program = None


def kernel(input_ids, token_emb, dinuc_emb, proj_W, proj_b, ln_gamma, ln_beta):
    global _program, last_run_results
    lut_hi, lut_lo = _build_lut(token_emb, dinuc_emb, proj_W, proj_b,
                                ln_gamma, ln_beta)

    in_maps = []
    for i in range(N_CORES):
        ids_rows = np.asarray(input_ids[i * ROWS_PER_CORE:(i + 1) * ROWS_PER_CORE],
                              dtype=np.int32)                    # [4, S]
        ids_next = np.full_like(ids_rows, 8)
        ids_next[:, :-1] = ids_rows[:, 1:]                       # sentinel at S-1
        # c-major layout: element (c, j) = flat position j*C + c
        to_cm = lambda a: np.ascontiguousarray(a.reshape(P, C).T)
        in_maps.append({
            "ids_a": to_cm(ids_rows.reshape(-1)),
            "ids_b": to_cm(ids_next.reshape(-1)),
            "lut_hi": lut_hi,
            "lut_lo": lut_lo,
        })

    if _program is None:
        _program = build_program()

    trace = os.environ.get("KERNEL_TRACE", "0") == "1"
    res = run_bass_kernel_spmd(_program, in_maps, list(range(N_CORES)),
                               trace=trace)
    last_run_results = res

    out = np.empty((B, S, H), dtype=np.float32)
    for i in range(N_CORES):
        out[i * ROWS_PER_CORE:(i + 1) * ROWS_PER_CORE] = (
            res.results[i]["out"].reshape(ROWS_PER_CORE, S, H))
    return out
